# revision 17
# baseline (speedup 1.0000x reference)
"""MixMOE forward on 8 Trainium2 NeuronCores.

Strategy (expert-parallel, sparse dispatch):
  - Host computes the tiny NaiveGate routing (logits -> top-2 -> softmax)
    in float64 (bitwise-stable ordering; the #2/#3 logit gap is >> fp32
    noise for this problem size, so routing matches the fp32 reference).
  - Tokens are gathered per expert (the "all-to-all dispatch"), transposed
    to [D, C] blocks, and shipped to the core that owns the expert
    (2 experts per core, 8 cores). Experts are ranked by token count:
    the 8 largest go to slot 0 (capacity C0), the 8 smallest to slot 1
    (capacity C1 <= C0), so padded columns cost less PE time. Weights are
    host-pre-tiled so every device DMA is fully contiguous.
  - Each core runs its expert FFNs as weights-stationary / tokens-moving
    fp16 matmuls (fp32 PSUM accumulation, ~4e-4 end-to-end rel-err):
        hT = gelu_tanh(W1_e^T @ xT + b1)   [H, C]
        yT = W2_e^T @ hT                   [D, C]
  - Host applies gate weights + b2 during the scatter-add combine
    (the "all-to-all return").
Compute is 8x less than the dense-equivalent reference (top-2 of 16).
"""

import sys

sys.path.insert(0, "/opt/trn_rl_repo")

import numpy as np

T, D, H, E, TOP_K, NCORES = 2048, 1024, 2048, 16, 2, 8
EPC = E // NCORES  # experts per core
DT8 = D // 128  # 8 d-tiles
HT16 = H // 128  # 16 h-tiles

_CACHE: dict = {}


def _build(Cs: tuple, dt_name: str, gelu_name: str = "Gelu_apprx_tanh"):
    """Build + finalize the per-core Bass program (SPMD across 8 cores).
    Cs[s] is the token capacity of expert-slot s; buffers are sized to
    C = max(Cs) and slot matmuls use their own moving-dim."""
    import concourse.bacc as bacc
    import concourse.mybir as mybir
    from concourse.tile import TileContext

    DT = getattr(mybir.dt, dt_name)
    f32 = mybir.dt.float32
    C = max(Cs)

    nc = bacc.Bacc("TRN2", target_bir_lowering=False)
    # Host-pre-tiled layouts: every DMA below is contiguous in HBM.
    xt = nc.dram_tensor("xt", [EPC, 128, DT8 * C], DT, kind="ExternalInput")
    w1 = nc.dram_tensor(
        "w1", [EPC, HT16 // 2, 128, 2, DT8, 128], DT, kind="ExternalInput"
    )
    w2 = nc.dram_tensor(
        "w2", [EPC, DT8 // 2, 128, 2, HT16, 128], DT, kind="ExternalInput"
    )
    b1 = nc.dram_tensor("b1", [128, EPC * HT16], f32, kind="ExternalInput")
    yt = nc.dram_tensor("yt", [EPC, DT8, 128, C], f32, kind="ExternalOutput")

    gelu = getattr(mybir.ActivationFunctionType, gelu_name)

    with TileContext(nc) as tc:
        with (
            tc.tile_pool(name="xpool", bufs=2) as xpool,
            tc.tile_pool(name="w1pool", bufs=6) as w1pool,
            tc.tile_pool(name="w2pool", bufs=4) as w2pool,
            tc.tile_pool(name="hpool", bufs=2) as hpool,
            tc.tile_pool(name="opool", bufs=3) as opool,
            tc.tile_pool(name="cpool", bufs=1) as cpool,
            tc.tile_pool(name="ps1", bufs=4, space="PSUM") as ps1,
            tc.tile_pool(name="ps2", bufs=4, space="PSUM") as ps2,
        ):
            # HAM pre-warm: keep PE busy during the initial weight/token DMAs
            # so the clock gate is at 8/8 when real matmuls start.
            warm = cpool.tile([128, 128], DT)
            nc.vector.memset(warm[:], 0.0)
            wps = ps1.tile([128, 128], f32, tag="ps1")
            for _ in range(28):
                nc.tensor.matmul(wps[:], warm[:], warm[:], start=True, stop=True)

            b1t = cpool.tile([128, EPC * HT16], f32)
            nc.gpsimd.dma_start(out=b1t, in_=b1[:, :])
            # Expert-0 tokens + odd W1 pairs ride the scalar HWDGE queue in
            # parallel with the sync weight stream; later experts prefetch
            # tokens on gpsimd during compute.
            xts = []
            for e in range(EPC):
                xte = xpool.tile([128, DT8, C], DT, tag="xt")
                xts.append(xte)
            # First d-chunk of expert-0 tokens on sync (ahead of the first
            # weight pair); the rest in parallel on scalar.
            nc.sync.dma_start(out=xts[0][:, 0, :], in_=xt[0][:, :C])
            nc.scalar.dma_start(
                out=xts[0].rearrange("p a b -> p (a b)")[:, C:],
                in_=xt[0][:, C:],
            )
            for e in range(EPC):
                Ce = Cs[e]
                xte = xts[e]
                hsb = hpool.tile([128, HT16, C], DT, tag="ht")
                # --- GEMM1 + gelu: hT[ht] = gelu(sum_dt W1[ht,:,dt].T @ xT[dt] + b1) ---
                w1t = None
                for ht in range(HT16):
                    if ht % 2 == 0:
                        w1t = w1pool.tile([128, 2, DT8, 128], DT, tag="w1")
                        nc.sync.dma_start(
                            out=w1t.rearrange("p a b c -> p (a b c)"),
                            in_=w1[e, ht // 2].rearrange("p a b c -> p (a b c)"),
                        )
                    acc = ps1.tile([128, Ce], f32, tag="ps1")
                    for dt in range(DT8):
                        nc.tensor.matmul(
                            acc[:],
                            w1t[:, ht % 2, dt, :],
                            xte[:, dt, :Ce],
                            start=(dt == 0),
                            stop=(dt == DT8 - 1),
                        )
                    nc.scalar.activation(
                        hsb[:, ht, :Ce],
                        acc[:],
                        gelu,
                        bias=b1t[:, e * HT16 + ht : e * HT16 + ht + 1],
                    )
                # Prefetch next expert's tokens before this expert's stores
                # occupy the gpsimd queue.
                if e + 1 < EPC:
                    nc.gpsimd.dma_start(
                        out=xts[e + 1].rearrange("p a b -> p (a b)"), in_=xt[e + 1]
                    )
                # --- GEMM2: yT[dt2] = sum_ht W2[dt2,:,ht].T @ hT[ht] ---
                w2t = None
                for dt2 in range(DT8):
                    if dt2 % 2 == 0:
                        w2t = w2pool.tile([128, 2, HT16, 128], DT, tag="w2")
                        nc.sync.dma_start(
                            out=w2t.rearrange("p a b c -> p (a b c)"),
                            in_=w2[e, dt2 // 2].rearrange("p a b c -> p (a b c)"),
                        )
                    acc2 = ps2.tile([128, Ce], f32, tag="ps2")
                    for ht in range(HT16):
                        nc.tensor.matmul(
                            acc2[:],
                            w2t[:, dt2 % 2, ht, :],
                            hsb[:, ht, :Ce],
                            start=(ht == 0),
                            stop=(ht == HT16 - 1),
                        )
                    ot = opool.tile([128, C], f32, tag="ot")
                    nc.vector.tensor_copy(ot[:, :Ce], acc2[:])
                    nc.gpsimd.dma_start(out=yt[e, dt2][:, :Ce], in_=ot[:, :Ce])
    nc.finalize()
    return nc


def _route(x: np.ndarray, gate_w: np.ndarray):
    """Host NaiveGate: fp64 logits -> stable top-2 -> softmax. Returns
    (top_idx [T,2] int, gate_score [T,2] f64)."""
    logits = x.astype(np.float64) @ gate_w.astype(np.float64)
    top_idx = np.argsort(-logits, axis=1, kind="stable")[:, :TOP_K]
    top_val = np.take_along_axis(logits, top_idx, axis=1)
    ex = np.exp(top_val - top_val.max(axis=1, keepdims=True))
    gate = ex / ex.sum(axis=1, keepdims=True)
    return top_idx, gate


def _run_device(nc, in_maps, trace=False, tmpdir=None):
    from concourse.bass_utils import run_bass_kernel_spmd

    return run_bass_kernel_spmd(
        nc, in_maps, core_ids=list(range(NCORES)), trace=trace, tmpdir=tmpdir
    )


DT_NAME = "float16"


def _npdt(dt_name):
    if dt_name == "bfloat16":
        import ml_dtypes

        return ml_dtypes.bfloat16
    return {"float16": np.float16, "float32r": np.float32}[dt_name]


def kernel(x, gate_w, W1, b1, W2, b2, _trace=False, _tmpdir=None):
    x = np.ascontiguousarray(np.asarray(x, dtype=np.float32))
    gate_w = np.asarray(gate_w, dtype=np.float32)
    W1 = np.asarray(W1, dtype=np.float32)
    b1 = np.asarray(b1, dtype=np.float32)
    W2 = np.asarray(W2, dtype=np.float32)
    b2 = np.asarray(b2, dtype=np.float32)

    top_idx, gate = _route(x, gate_w)

    idx_e = [np.where(top_idx == e)[0] for e in range(E)]
    gat_e = [gate[top_idx == e] for e in range(E)]
    counts = np.array([len(i) for i in idx_e])

    # Slot assignment: 8 largest experts -> slot 0, 8 smallest -> slot 1.
    order = np.argsort(-counts, kind="stable")
    assign = [[int(order[c]), int(order[NCORES + c])] for c in range(NCORES)]
    r8 = lambda v: -(-int(v) // 8) * 8
    Cs = (r8(counts[order[0]]), r8(counts[order[NCORES]]))
    C = max(Cs)

    npdt = _npdt(DT_NAME)
    key = (Cs, DT_NAME)
    if key not in _CACHE:
        _CACHE[key] = _build(Cs, DT_NAME)
    nc = _CACHE[key]

    in_maps = []
    for core in range(NCORES):
        es = assign[core]
        xt = np.zeros((EPC, 128, DT8, C), npdt)
        for sl in range(EPC):
            ids = idx_e[es[sl]]
            # [C_e, D] -> [D, C_e] -> [dt, p, C_e] -> [p, dt, C_e]
            xg = x[ids].T.reshape(DT8, 128, len(ids)).transpose(1, 0, 2)
            xt[sl, :, :, : len(ids)] = xg
        xt = xt.reshape(EPC, 128, DT8 * C)
        # W1[e,d,h] -> [e, hp, p, pm, dt, m];  W2[e,h,d] -> [e, dp, p, pm, ht, m]
        w1p = np.ascontiguousarray(
            W1[es]
            .reshape(EPC, DT8, 128, HT16 // 2, 2, 128)
            .transpose(0, 3, 2, 4, 1, 5),
            dtype=npdt,
        )
        w2p = np.ascontiguousarray(
            W2[es]
            .reshape(EPC, HT16, 128, DT8 // 2, 2, 128)
            .transpose(0, 3, 2, 4, 1, 5),
            dtype=npdt,
        )
        in_maps.append(
            {
                "xt": xt,
                "w1": w1p,
                "w2": w2p,
                "b1": np.ascontiguousarray(b1[es].reshape(EPC * HT16, 128).T),
            }
        )

    res = _run_device(nc, in_maps, trace=_trace, tmpdir=_tmpdir)

    out = np.zeros((T, D), np.float32)
    for core in range(NCORES):
        for sl in range(EPC):
            e = assign[core][sl]
            ids = idx_e[e]
            if len(ids) == 0:
                continue
            # yt: [dt2, 128, C] -> [D, C_e] -> [C_e, D]
            y = res.results[core]["yt"][sl].reshape(D, C)[:, : len(ids)].T
            out[ids] += (gat_e[e][:, None] * (y + b2[e][None, :])).astype(
                np.float32
            )

    if _trace:
        return out, res
    return out


# revision 18
# speedup vs baseline: 1.0029x; 1.0029x over previous
"""MixMOE forward on 8 Trainium2 NeuronCores.

Strategy (expert-parallel, sparse dispatch):
  - Host computes the tiny NaiveGate routing (logits -> top-2 -> softmax)
    in float64 (bitwise-stable ordering; the #2/#3 logit gap is >> fp32
    noise for this problem size, so routing matches the fp32 reference).
  - Tokens are gathered per expert (the "all-to-all dispatch"), transposed
    to [D, C] blocks, and shipped to the core that owns the expert
    (2 experts per core, 8 cores). Experts are ranked by token count:
    the 8 largest go to slot 0 (capacity C0), the 8 smallest to slot 1
    (capacity C1 <= C0), so padded columns cost less PE time. Weights are
    host-pre-tiled so every device DMA is fully contiguous.
  - Each core runs its expert FFNs as weights-stationary / tokens-moving
    fp16 matmuls (fp32 PSUM accumulation, ~4e-4 end-to-end rel-err):
        hT = gelu_tanh(W1_e^T @ xT + b1)   [H, C]
        yT = W2_e^T @ hT                   [D, C]
  - Host applies gate weights + b2 during the scatter-add combine
    (the "all-to-all return").
Compute is 8x less than the dense-equivalent reference (top-2 of 16).
"""

import sys

sys.path.insert(0, "/opt/trn_rl_repo")

import numpy as np

T, D, H, E, TOP_K, NCORES = 2048, 1024, 2048, 16, 2, 8
EPC = E // NCORES  # experts per core
DT8 = D // 128  # 8 d-tiles
HT16 = H // 128  # 16 h-tiles

_CACHE: dict = {}


def _build(Cs: tuple, dt_name: str, gelu_name: str = "Gelu_apprx_tanh"):
    """Build + finalize the per-core Bass program (SPMD across 8 cores).
    Cs[s] is the token capacity of expert-slot s; buffers are sized to
    C = max(Cs) and slot matmuls use their own moving-dim."""
    import concourse.bacc as bacc
    import concourse.mybir as mybir
    from concourse.tile import TileContext

    DT = getattr(mybir.dt, dt_name)
    f32 = mybir.dt.float32
    C = max(Cs)

    nc = bacc.Bacc("TRN2", target_bir_lowering=False)
    # Host-pre-tiled layouts: every DMA below is contiguous in HBM.
    xt = nc.dram_tensor("xt", [EPC, 128, DT8 * C], DT, kind="ExternalInput")
    w1 = nc.dram_tensor(
        "w1", [EPC, HT16 // 2, 128, 2, DT8, 128], DT, kind="ExternalInput"
    )
    w2 = nc.dram_tensor(
        "w2", [EPC, DT8 // 2, 128, 2, HT16, 128], DT, kind="ExternalInput"
    )
    b1 = nc.dram_tensor("b1", [128, EPC * HT16], f32, kind="ExternalInput")
    yt = nc.dram_tensor("yt", [EPC, DT8, 128, C], f32, kind="ExternalOutput")

    gelu = getattr(mybir.ActivationFunctionType, gelu_name)

    with TileContext(nc) as tc:
        with (
            tc.tile_pool(name="xpool", bufs=2) as xpool,
            tc.tile_pool(name="w1pool", bufs=6) as w1pool,
            tc.tile_pool(name="w2pool", bufs=4) as w2pool,
            tc.tile_pool(name="hpool", bufs=2) as hpool,
            tc.tile_pool(name="opool", bufs=3) as opool,
            tc.tile_pool(name="cpool", bufs=1) as cpool,
            tc.tile_pool(name="ps1", bufs=4, space="PSUM") as ps1,
            tc.tile_pool(name="ps2", bufs=4, space="PSUM") as ps2,
        ):
            # HAM pre-warm: keep PE busy during the initial weight/token DMAs
            # so the clock gate is at 8/8 when real matmuls start.
            warm = cpool.tile([128, 128], DT)
            nc.vector.memset(warm[:], 0.0)
            wps = ps1.tile([128, 128], f32, tag="ps1")
            for _ in range(40):
                nc.tensor.matmul(wps[:], warm[:], warm[:], start=True, stop=True)

            b1t = cpool.tile([128, EPC * HT16], f32)
            nc.gpsimd.dma_start(out=b1t, in_=b1[:, :])
            # Expert-0 tokens + odd W1 pairs ride the scalar HWDGE queue in
            # parallel with the sync weight stream; later experts prefetch
            # tokens on gpsimd during compute.
            xts = []
            for e in range(EPC):
                xte = xpool.tile([128, DT8, C], DT, tag="xt")
                xts.append(xte)
            nc.scalar.dma_start(
                out=xts[0].rearrange("p a b -> p (a b)"), in_=xt[0]
            )
            for e in range(EPC):
                Ce = Cs[e]
                xte = xts[e]
                hsb = hpool.tile([128, HT16, C], DT, tag="ht")
                # --- GEMM1 + gelu: hT[ht] = gelu(sum_dt W1[ht,:,dt].T @ xT[dt] + b1) ---
                w1t = None
                for ht in range(HT16):
                    if ht % 2 == 0:
                        w1t = w1pool.tile([128, 2, DT8, 128], DT, tag="w1")
                        nc.sync.dma_start(
                            out=w1t.rearrange("p a b c -> p (a b c)"),
                            in_=w1[e, ht // 2].rearrange("p a b c -> p (a b c)"),
                        )
                    acc = ps1.tile([128, Ce], f32, tag="ps1")
                    for dt in range(DT8):
                        nc.tensor.matmul(
                            acc[:],
                            w1t[:, ht % 2, dt, :],
                            xte[:, dt, :Ce],
                            start=(dt == 0),
                            stop=(dt == DT8 - 1),
                        )
                    nc.scalar.activation(
                        hsb[:, ht, :Ce],
                        acc[:],
                        gelu,
                        bias=b1t[:, e * HT16 + ht : e * HT16 + ht + 1],
                    )
                # Prefetch next expert's tokens before this expert's stores
                # occupy the gpsimd queue.
                if e + 1 < EPC:
                    nc.gpsimd.dma_start(
                        out=xts[e + 1].rearrange("p a b -> p (a b)"), in_=xt[e + 1]
                    )
                # --- GEMM2: yT[dt2] = sum_ht W2[dt2,:,ht].T @ hT[ht] ---
                w2t = None
                for dt2 in range(DT8):
                    if dt2 % 2 == 0:
                        w2t = w2pool.tile([128, 2, HT16, 128], DT, tag="w2")
                        nc.sync.dma_start(
                            out=w2t.rearrange("p a b c -> p (a b c)"),
                            in_=w2[e, dt2 // 2].rearrange("p a b c -> p (a b c)"),
                        )
                    acc2 = ps2.tile([128, Ce], f32, tag="ps2")
                    for ht in range(HT16):
                        nc.tensor.matmul(
                            acc2[:],
                            w2t[:, dt2 % 2, ht, :],
                            hsb[:, ht, :Ce],
                            start=(ht == 0),
                            stop=(ht == HT16 - 1),
                        )
                    ot = opool.tile([128, C], f32, tag="ot")
                    nc.vector.tensor_copy(ot[:, :Ce], acc2[:])
                    nc.gpsimd.dma_start(out=yt[e, dt2][:, :Ce], in_=ot[:, :Ce])
    nc.finalize()
    return nc


def _route(x: np.ndarray, gate_w: np.ndarray):
    """Host NaiveGate: fp64 logits -> stable top-2 -> softmax. Returns
    (top_idx [T,2] int, gate_score [T,2] f64)."""
    logits = x.astype(np.float64) @ gate_w.astype(np.float64)
    top_idx = np.argsort(-logits, axis=1, kind="stable")[:, :TOP_K]
    top_val = np.take_along_axis(logits, top_idx, axis=1)
    ex = np.exp(top_val - top_val.max(axis=1, keepdims=True))
    gate = ex / ex.sum(axis=1, keepdims=True)
    return top_idx, gate


def _run_device(nc, in_maps, trace=False, tmpdir=None):
    from concourse.bass_utils import run_bass_kernel_spmd

    return run_bass_kernel_spmd(
        nc, in_maps, core_ids=list(range(NCORES)), trace=trace, tmpdir=tmpdir
    )


DT_NAME = "float16"


def _npdt(dt_name):
    if dt_name == "bfloat16":
        import ml_dtypes

        return ml_dtypes.bfloat16
    return {"float16": np.float16, "float32r": np.float32}[dt_name]


def kernel(x, gate_w, W1, b1, W2, b2, _trace=False, _tmpdir=None):
    x = np.ascontiguousarray(np.asarray(x, dtype=np.float32))
    gate_w = np.asarray(gate_w, dtype=np.float32)
    W1 = np.asarray(W1, dtype=np.float32)
    b1 = np.asarray(b1, dtype=np.float32)
    W2 = np.asarray(W2, dtype=np.float32)
    b2 = np.asarray(b2, dtype=np.float32)

    top_idx, gate = _route(x, gate_w)

    idx_e = [np.where(top_idx == e)[0] for e in range(E)]
    gat_e = [gate[top_idx == e] for e in range(E)]
    counts = np.array([len(i) for i in idx_e])

    # Slot assignment: 8 largest experts -> slot 0, 8 smallest -> slot 1.
    order = np.argsort(-counts, kind="stable")
    assign = [[int(order[c]), int(order[NCORES + c])] for c in range(NCORES)]
    r8 = lambda v: -(-int(v) // 8) * 8
    Cs = (r8(counts[order[0]]), r8(counts[order[NCORES]]))
    C = max(Cs)

    npdt = _npdt(DT_NAME)
    key = (Cs, DT_NAME)
    if key not in _CACHE:
        _CACHE[key] = _build(Cs, DT_NAME)
    nc = _CACHE[key]

    in_maps = []
    for core in range(NCORES):
        es = assign[core]
        xt = np.zeros((EPC, 128, DT8, C), npdt)
        for sl in range(EPC):
            ids = idx_e[es[sl]]
            # [C_e, D] -> [D, C_e] -> [dt, p, C_e] -> [p, dt, C_e]
            xg = x[ids].T.reshape(DT8, 128, len(ids)).transpose(1, 0, 2)
            xt[sl, :, :, : len(ids)] = xg
        xt = xt.reshape(EPC, 128, DT8 * C)
        # W1[e,d,h] -> [e, hp, p, pm, dt, m];  W2[e,h,d] -> [e, dp, p, pm, ht, m]
        w1p = np.ascontiguousarray(
            W1[es]
            .reshape(EPC, DT8, 128, HT16 // 2, 2, 128)
            .transpose(0, 3, 2, 4, 1, 5),
            dtype=npdt,
        )
        w2p = np.ascontiguousarray(
            W2[es]
            .reshape(EPC, HT16, 128, DT8 // 2, 2, 128)
            .transpose(0, 3, 2, 4, 1, 5),
            dtype=npdt,
        )
        in_maps.append(
            {
                "xt": xt,
                "w1": w1p,
                "w2": w2p,
                "b1": np.ascontiguousarray(b1[es].reshape(EPC * HT16, 128).T),
            }
        )

    res = _run_device(nc, in_maps, trace=_trace, tmpdir=_tmpdir)

    out = np.zeros((T, D), np.float32)
    for core in range(NCORES):
        for sl in range(EPC):
            e = assign[core][sl]
            ids = idx_e[e]
            if len(ids) == 0:
                continue
            # yt: [dt2, 128, C] -> [D, C_e] -> [C_e, D]
            y = res.results[core]["yt"][sl].reshape(D, C)[:, : len(ids)].T
            out[ids] += (gat_e[e][:, None] * (y + b2[e][None, :])).astype(
                np.float32
            )

    if _trace:
        return out, res
    return out
